# revision 18
# baseline (speedup 1.0000x reference)
"""Trainium2 Bass kernel for nn_FGNet (gnn_message_passing), v6.

bf16 + PE array tiling.  Blocks are processed in PAIRS:
  - mm1 (K=64): row-tiled 64x128 -- block 2j's feats/W live on SBUF
    partitions 0:64 (array tile T0), block 2j+1's on 64:128 (T8); the
    two transforms run CONCURRENTLY in the PE array.
  - mm2 (M=64): col-tiled 128x64 -- block 2j's messages land on PSUM
    partitions 0:64 (T0), block 2j+1's on 64:128 (T1), also concurrent.
    One [128,768] cast then drains BOTH blocks' messages.
Bias is applied by the Act engine (bias operand of the relu activation).
Wave-pipelined emission: PE alternates [mm1-pair of wave v] and
[mm2 x6 of wave v-1] so products always have a full wave to land.

Host side: id sort, gather, packing; overflow edges (>256 per id) and
the final segment-sum are computed on host.
"""

import numpy as np

_BLK = 256          # edge slots per block
_NCORES = 8
_GP = 2             # block-pairs per DMA group (4 blocks)

_prog_cache = {}


def _bf16(x):
    import jax.numpy as jnp
    return np.asarray(jnp.asarray(x, dtype=jnp.bfloat16))


def _build_program(B):
    """Device program: B blocks per core, processed as B/2 pairs."""
    import concourse.mybir as mybir
    import concourse.tile as tile
    from concourse import bacc

    F32 = mybir.dt.float32
    BF16 = mybir.dt.bfloat16
    Relu = mybir.ActivationFunctionType.Relu
    Copy = mybir.ActivationFunctionType.Copy

    assert B % 2 == 0
    B2 = B // 2

    nc = bacc.Bacc()
    # pk rows 0:64 = even block, rows 64:128 = odd block of the pair
    # cols 0:768 feats (col = i*256+e), 768:896 = W
    pk = nc.declare_dram_parameter("pk", [128, B2, 896], BF16, isOutput=False)
    bia = nc.declare_dram_parameter("bia", [128, B], F32, isOutput=False)
    hot = nc.declare_dram_parameter("hot", [128, B, 192], BF16, isOutput=False)
    msgs = nc.declare_dram_parameter("msgs", [128, B2, 768], BF16, isOutput=True)

    groups = [(0, 1)]          # small first group -> fast pipeline start
    g0 = 1
    while g0 < B2:
        g = min(_GP, B2 - g0)
        groups.append((g0, g))
        g0 += g
    grp_of = {}
    for gi, (g0, g) in enumerate(groups):
        for j in range(g0, g0 + g):
            grp_of[j] = gi

    with tile.TileContext(nc) as tc:
        with (
            tc.tile_pool(name="const", bufs=1) as const,
            tc.tile_pool(name="work", bufs=3) as work,
            tc.tile_pool(name="blk", bufs=6) as blk,
            tc.tile_pool(name="ps1p", bufs=1, space="PSUM") as ps1p,
            tc.tile_pool(name="ps2p", bufs=2, space="PSUM") as ps2p,
        ):
            bt = const.tile([128, B], F32, name="bt")
            nc.sync.dma_start(out=bt[:], in_=bia[:])

            ftt, htt, mt = {}, {}, {}
            pt = {}

            def load_group(gi):
                g0, g = groups[gi]
                ft = work.tile([128, g, 896], BF16, name="ft", tag="ft")
                nc.sync.dma_start(out=ft[:], in_=pk[:, g0:g0 + g, :])
                ht = work.tile([128, 2 * g, 192], BF16, name="ht", tag="ht")
                nc.sync.dma_start(out=ht[:], in_=hot[:, 2 * g0:2 * (g0 + g), :])
                mt[gi] = work.tile([128, g, 768], BF16, name="m", tag="m")
                ftt[gi], htt[gi] = ft, ht

            def emit_front(j):
                """Row-tiled mm1 pair + relus + products for pair j."""
                gi = grp_of[j]
                if j == 0:
                    load_group(0)
                g0, _ = groups[gi]
                if j == g0 and gi + 1 < len(groups):
                    load_group(gi + 1)
                ft = ftt[gi]
                jj = j - g0
                ps1a = ps1p.tile([128, 2, 512], F32, name="ps1a", tag="ps1a")
                ps1b = ps1p.tile([128, 2, 512], F32, name="ps1b", tag="ps1b")
                # interleave T0/T8 so the two array tiles run concurrently
                nc.tensor.matmul(out=ps1a[:, 0, 0:384],
                                 lhsT=ft[0:64, jj, 768:896],
                                 rhs=ft[0:64, jj, 0:384],
                                 start=True, stop=True)
                nc.tensor.matmul(out=ps1b[:, 0, 0:384],
                                 lhsT=ft[64:128, jj, 768:896],
                                 rhs=ft[64:128, jj, 0:384],
                                 start=True, stop=True)
                nc.tensor.matmul(out=ps1a[:, 1, 0:384],
                                 lhsT=ft[0:64, jj, 768:896],
                                 rhs=ft[0:64, jj, 384:768],
                                 start=True, stop=True)
                nc.tensor.matmul(out=ps1b[:, 1, 0:384],
                                 lhsT=ft[64:128, jj, 768:896],
                                 rhs=ft[64:128, jj, 384:768],
                                 start=True, stop=True)
                for h, ps1 in ((0, ps1a), (1, ps1b)):
                    k = 2 * j + h
                    t = blk.tile([128, 768], BF16, name="t", tag=f"t{h}")
                    nc.scalar.activation(
                        out=t[:].rearrange("r (s c) -> r s c", s=2, c=384),
                        in_=ps1[:, :, 0:384],
                        func=Relu, bias=bt[:, k:k + 1], scale=1.0)
                    p2 = blk.tile([128, 256], BF16, name="p2", tag=f"p2{h}")
                    nc.vector.tensor_mul(out=p2[:], in0=t[:, 0:256],
                                         in1=t[:, 256:512])
                    p1 = blk.tile([128, 256], BF16, name="p1", tag=f"p1{h}")
                    peng = nc.gpsimd if h == 1 else nc.vector
                    peng.tensor_mul(out=p1[:], in0=t[:, 0:256],
                                    in1=t[:, 512:768])
                    p0 = blk.tile([128, 256], BF16, name="p0", tag=f"p0{h}")
                    nc.gpsimd.tensor_mul(out=p0[:], in0=t[:, 256:512],
                                         in1=t[:, 512:768])
                    pt[k] = (p0, p1, p2)

            def emit_back(j):
                """Col-tiled mm2 x6 + cast + out-DMA for pair j."""
                gi = grp_of[j]
                g0, g = groups[gi]
                jj = j - g0
                ht = htt[gi]
                ps2 = ps2p.tile([128, 3, 256], F32, name="ps2", tag="ps2")
                for i in (2, 1, 0):
                    for h in (0, 1):
                        k = 2 * j + h
                        pi = pt[k][i]
                        nc.tensor.matmul(
                            out=ps2[64 * h:64 * (h + 1), i, :],
                            lhsT=ht[:, 2 * jj + h, 64 * i:64 * (i + 1)],
                            rhs=pi[:],
                            start=True, stop=True)
                pt.pop(2 * j)
                pt.pop(2 * j + 1)
                mk = mt[gi][:, jj, :]
                ps2f = ps2[:].rearrange("l i e -> l (i e)")
                nc.scalar.activation(out=mk[:, 0:256], in_=ps2f[:, 0:256],
                                     func=Copy, bias=0.0, scale=1.0)
                nc.vector.tensor_copy(out=mk[:, 256:768], in_=ps2f[:, 256:768])
                if jj == g - 1:
                    nc.sync.dma_start(out=msgs[:, g0:g0 + g, :],
                                      in_=mt[gi][:])

            for v in range(B2 + 1):
                if v < B2:
                    emit_front(v)
                if v >= 1:
                    emit_back(v - 1)
    nc.finalize()
    return nc


def _get_program(B):
    if B not in _prog_cache:
        _prog_cache[B] = _build_program(B)
    return _prog_cache[B]


def _prepare(x, nodes, fact, params, bias_p, ho_params, ho_bias):
    """Host-side: sort by id, build per-block packed arrays."""
    N, L = nodes.shape
    E = fact.shape[0]
    R = params.shape[2]
    NP = params.shape[0]           # 169
    MA = int(round(NP ** 0.5))     # 13

    ids = (x[fact[:, 0], 1] * MA + x[fact[:, 0], 2]).astype(np.int64)   # [E]
    perm = np.argsort(ids, kind="stable")
    ids_s = ids[perm]
    fact_s = fact[perm].astype(np.int64)                                 # [E,3]

    counts = np.bincount(ids_s, minlength=NP)                            # [NP]
    # one 256-block per id on device; overflow edges (count > 256, ~1.5%)
    # are computed host-side so the device program stays uniform
    dev_counts = np.minimum(counts, _BLK)
    NB = NP                                                              # 169
    B = (NB + _NCORES - 1) // _NCORES
    if B % 2:
        B += 1
    NB8 = B * _NCORES
    blk_ids = np.concatenate([np.arange(NP), np.zeros(NB8 - NB, np.int64)])

    # slot -> sorted-edge-position map (-1 = padding)
    off = np.concatenate([[0], np.cumsum(counts)])
    jloc = np.tile(np.arange(_BLK), NB)
    t_of = np.repeat(np.arange(NP), _BLK)
    src = np.where(jloc < dev_counts[t_of], off[t_of] + jloc, -1)
    src = np.concatenate([src, np.full((NB8 - NB) * _BLK, -1, np.int64)])
    valid = src >= 0

    # overflow edge positions (in sorted order)
    ov_mask = np.zeros(E, bool)
    for tid in np.nonzero(counts > _BLK)[0]:
        ov_mask[off[tid] + _BLK:off[tid + 1]] = True
    ov_pos = np.nonzero(ov_mask)[0]

    # gather features per slot
    nf = nodes[fact_s]                                                   # [E,3,L]
    featp = np.zeros((NB8 * _BLK, 3, L), np.float32)
    featp[valid] = nf[src[valid]]

    # pack pk [8][128, B/2, 896]: even block rows 0:64, odd rows 64:128
    fw = np.zeros((NB8, 64, 896), np.float32)
    fw[:, :, 0:768] = (
        featp.reshape(NB8, _BLK, 3, L).transpose(0, 3, 2, 1)
        .reshape(NB8, 64, 768)
    )
    fw[:, :, 768:896] = params[blk_ids].astype(np.float32)               # W
    fw = _bf16(fw)
    B2 = B // 2
    pk = (fw.reshape(_NCORES, B2, 2, 64, 896).transpose(0, 2, 3, 1, 4)
          .reshape(_NCORES, 128, B2, 896))

    bia = bias_p[blk_ids, 0].astype(np.float32)                          # [NB8,R]
    bia = bia.reshape(_NCORES, B, R).transpose(0, 2, 1)                  # [8,R,B]

    hot = (
        ho_params[:, blk_ids].astype(np.float32).transpose(1, 2, 0, 3)
        .reshape(NB8, R, 3 * L)
    )
    hot = _bf16(hot).reshape(_NCORES, B, R, 192).transpose(0, 2, 1, 3)

    # host path for overflow edges
    msg_ov = None
    if ov_pos.size:
        f_ov = fact_s[ov_pos]                                            # [V,3]
        id_ov = ids_s[ov_pos]                                            # [V]
        W_ov = params[id_ov].astype(np.float32)                          # [V,L,R]
        b_ov = bias_p[id_ov, 0].astype(np.float32)                       # [V,R]
        rn = nodes[f_ov].astype(np.float32)                              # [V,3,L]
        tv = np.maximum(np.einsum('vil,vlr->vir', rn, W_ov) + b_ov[:, None], 0)
        msg_ov = np.empty((ov_pos.size, 3, L), np.float32)
        W2_ov = ho_params[:, id_ov].astype(np.float32)                   # [3,V,R,L]
        for i in range(3):
            j, k2 = [(1, 2), (0, 2), (0, 1)][i]
            pv = tv[:, j] * tv[:, k2]                                    # [V,R]
            msg_ov[:, i] = np.einsum('vr,vrl->vl', pv, W2_ov[i])

    return dict(pk=np.ascontiguousarray(pk),
                bia=np.ascontiguousarray(bia),
                hot=np.ascontiguousarray(hot),
                B=B, NB8=NB8, src=src, valid=valid, fact_s=fact_s,
                ids_s=ids_s, N=N, E=E, L=L, ov_pos=ov_pos, msg_ov=msg_ov)


def _postprocess(msgs_all, prep, ho_bias):
    """Decode per-slot messages, add host-side b2, segment-sum into node_msg."""
    NB8, N, E, L = prep["NB8"], prep["N"], prep["E"], prep["L"]
    src, valid, fact_s, ids_s = prep["src"], prep["valid"], prep["fact_s"], prep["ids_s"]
    B = prep["B"]
    B2 = B // 2
    # msgs_all [8][128, B2, 768] -> [NB8, 64, 768]: row l, col = i*256 + e
    m = (msgs_all.astype(np.float32)
         .reshape(_NCORES, 2, 64, B2, 768).transpose(0, 3, 1, 2, 4)
         .reshape(NB8, 64, 768))
    slots = (
        m.reshape(NB8, 64, 3, _BLK).transpose(0, 3, 2, 1)
        .reshape(NB8 * _BLK, 3, 64)
    )
    msg_e = np.empty((E, 3, L), np.float32)
    msg_e[src[valid]] = slots[valid]
    if prep["msg_ov"] is not None:
        msg_e[prep["ov_pos"]] = prep["msg_ov"]

    # fold in the second bias (linear in the segment-sum)
    msg_e += ho_bias[:, ids_s, 0].astype(np.float32).transpose(1, 0, 2)  # [E,3,L]

    idx_all = fact_s.T.reshape(-1)                                       # [3E]
    val_all = msg_e.transpose(1, 0, 2).reshape(-1, L)                    # [3E,L]
    order = np.argsort(idx_all, kind="stable")
    idx_sorted = idx_all[order]
    val_sorted = val_all[order]
    uniq, starts = np.unique(idx_sorted, return_index=True)
    sums = np.add.reduceat(val_sorted, starts, axis=0)
    out = np.zeros((N, L), np.float32)
    out[uniq] = sums
    return out


def _run_device(prep, trace=False, trace_kwargs=None):
    from concourse.bass_utils import run_bass_kernel_spmd

    B = prep["B"]
    nc = _get_program(B)
    in_maps = []
    for c in range(_NCORES):
        in_maps.append({
            "pk": prep["pk"][c],
            "bia": prep["bia"][c],
            "hot": prep["hot"][c],
        })
    kwargs = {}
    if trace:
        kwargs["trace"] = True
        if trace_kwargs:
            kwargs.update(trace_kwargs)
    res = run_bass_kernel_spmd(nc, in_maps, list(range(_NCORES)), **kwargs)
    msgs_all = np.stack([np.asarray(res.results[c]["msgs"]).astype(np.float32)
                         for c in range(_NCORES)], axis=0)
    return msgs_all, res


def kernel(x, nodes, fact, fact_dim, params, bias_p, ho_params, ho_bias,
           _trace=False, _trace_kwargs=None):
    x = np.asarray(x)
    nodes = np.asarray(nodes, dtype=np.float32)
    fact = np.asarray(fact)
    params = np.asarray(params)
    bias_p = np.asarray(bias_p)
    ho_params = np.asarray(ho_params)
    ho_bias = np.asarray(ho_bias)

    prep = _prepare(x, nodes, fact, params, bias_p, ho_params, ho_bias)
    msgs_all, res = _run_device(prep, trace=_trace, trace_kwargs=_trace_kwargs)
    out = _postprocess(msgs_all, prep, ho_bias)
    kernel.last_results = res
    return out


# revision 22
# speedup vs baseline: 1.0091x; 1.0091x over previous
"""Trainium2 Bass kernel for nn_FGNet (gnn_message_passing), v6.

bf16 + PE array tiling.  Blocks are processed in PAIRS:
  - mm1 (K=64): row-tiled 64x128 -- block 2j's feats/W live on SBUF
    partitions 0:64 (array tile T0), block 2j+1's on 64:128 (T8); the
    two transforms run CONCURRENTLY in the PE array.
  - mm2 (M=64): col-tiled 128x64 -- block 2j's messages land on PSUM
    partitions 0:64 (T0), block 2j+1's on 64:128 (T1), also concurrent.
    One [128,768] cast then drains BOTH blocks' messages.
Bias is applied by the Act engine (bias operand of the relu activation).
Wave-pipelined emission: PE alternates [mm1-pair of wave v] and
[mm2 x6 of wave v-1] so products always have a full wave to land.

Host side: id sort, gather, packing; overflow edges (>256 per id) and
the final segment-sum are computed on host.
"""

import numpy as np

_BLK = 256          # edge slots per block
_NCORES = 8
_GP = 2             # block-pairs per DMA group (4 blocks)

_prog_cache = {}


def _bf16(x):
    import jax.numpy as jnp
    return np.asarray(jnp.asarray(x, dtype=jnp.bfloat16))


def _build_program(B):
    """Device program: B blocks per core, processed as B/2 pairs."""
    import concourse.mybir as mybir
    import concourse.tile as tile
    from concourse import bacc

    F32 = mybir.dt.float32
    BF16 = mybir.dt.bfloat16
    Relu = mybir.ActivationFunctionType.Relu
    Copy = mybir.ActivationFunctionType.Copy

    assert B % 2 == 0
    B2 = B // 2

    nc = bacc.Bacc()
    # pk rows 0:64 = even block, rows 64:128 = odd block of the pair
    # cols 0:768 feats (col = i*256+e), 768:896 = W
    pk = nc.declare_dram_parameter("pk", [128, B2, 896], BF16, isOutput=False)
    bia = nc.declare_dram_parameter("bia", [128, B], F32, isOutput=False)
    hot = nc.declare_dram_parameter("hot", [128, B, 192], BF16, isOutput=False)
    msgs = nc.declare_dram_parameter("msgs", [128, B2, 768], BF16, isOutput=True)

    groups = []
    g0 = 0
    while g0 < B2:
        g = min(_GP, B2 - g0)
        groups.append((g0, g))
        g0 += g
    grp_of = {}
    for gi, (g0, g) in enumerate(groups):
        for j in range(g0, g0 + g):
            grp_of[j] = gi

    with tile.TileContext(nc) as tc:
        with (
            tc.tile_pool(name="const", bufs=1) as const,
            tc.tile_pool(name="work", bufs=3) as work,
            tc.tile_pool(name="blk", bufs=4) as blk,
            tc.tile_pool(name="ps1p", bufs=1, space="PSUM") as ps1p,
            tc.tile_pool(name="ps2p", bufs=2, space="PSUM") as ps2p,
        ):
            bt = const.tile([128, B], F32, name="bt")
            nc.sync.dma_start(out=bt[:], in_=bia[:])

            ftt, htt, mt = {}, {}, {}
            pt = {}

            def load_group(gi):
                g0, g = groups[gi]
                ft = work.tile([128, g, 896], BF16, name="ft", tag="ft")
                nc.sync.dma_start(out=ft[:], in_=pk[:, g0:g0 + g, :])
                ht = work.tile([128, 2 * g, 192], BF16, name="ht", tag="ht")
                nc.sync.dma_start(out=ht[:], in_=hot[:, 2 * g0:2 * (g0 + g), :])
                mt[gi] = work.tile([128, g, 768], BF16, name="m", tag="m")
                ftt[gi], htt[gi] = ft, ht

            def emit_front(j):
                """Row-tiled mm1 pair + relus + products for pair j."""
                gi = grp_of[j]
                if j == 0:
                    load_group(0)
                g0, _ = groups[gi]
                if j == g0 and gi + 1 < len(groups):
                    load_group(gi + 1)
                ft = ftt[gi]
                jj = j - g0
                ps1a = ps1p.tile([128, 2, 512], F32, name="ps1a", tag="ps1a")
                ps1b = ps1p.tile([128, 2, 512], F32, name="ps1b", tag="ps1b")
                # interleave T0/T8 so the two array tiles run concurrently
                nc.tensor.matmul(out=ps1a[:, 0, 0:384],
                                 lhsT=ft[0:64, jj, 768:896],
                                 rhs=ft[0:64, jj, 0:384],
                                 start=True, stop=True)
                nc.tensor.matmul(out=ps1b[:, 0, 0:384],
                                 lhsT=ft[64:128, jj, 768:896],
                                 rhs=ft[64:128, jj, 0:384],
                                 start=True, stop=True)
                nc.tensor.matmul(out=ps1a[:, 1, 0:384],
                                 lhsT=ft[0:64, jj, 768:896],
                                 rhs=ft[0:64, jj, 384:768],
                                 start=True, stop=True)
                nc.tensor.matmul(out=ps1b[:, 1, 0:384],
                                 lhsT=ft[64:128, jj, 768:896],
                                 rhs=ft[64:128, jj, 384:768],
                                 start=True, stop=True)
                for h, ps1 in ((0, ps1a), (1, ps1b)):
                    k = 2 * j + h
                    t = blk.tile([128, 768], BF16, name="t", tag=f"t{h}")
                    nc.scalar.activation(
                        out=t[:].rearrange("r (s c) -> r s c", s=2, c=384),
                        in_=ps1[:, :, 0:384],
                        func=Relu, bias=bt[:, k:k + 1], scale=1.0)
                    p2 = blk.tile([128, 256], BF16, name="p2", tag=f"p2{h}")
                    nc.vector.tensor_mul(out=p2[:], in0=t[:, 0:256],
                                         in1=t[:, 256:512])
                    p1 = blk.tile([128, 256], BF16, name="p1", tag=f"p1{h}")
                    nc.vector.tensor_mul(out=p1[:], in0=t[:, 0:256],
                                         in1=t[:, 512:768])
                    p0 = blk.tile([128, 256], BF16, name="p0", tag=f"p0{h}")
                    nc.gpsimd.tensor_mul(out=p0[:], in0=t[:, 256:512],
                                         in1=t[:, 512:768])
                    pt[k] = (p0, p1, p2)

            def emit_back(j):
                """Col-tiled mm2 x6 + cast + out-DMA for pair j."""
                gi = grp_of[j]
                g0, g = groups[gi]
                jj = j - g0
                ht = htt[gi]
                ps2 = ps2p.tile([128, 3, 256], F32, name="ps2", tag="ps2")
                for i in (2, 1, 0):
                    for h in (0, 1):
                        k = 2 * j + h
                        pi = pt[k][i]
                        nc.tensor.matmul(
                            out=ps2[64 * h:64 * (h + 1), i, :],
                            lhsT=ht[:, 2 * jj + h, 64 * i:64 * (i + 1)],
                            rhs=pi[:],
                            start=True, stop=True)
                pt.pop(2 * j)
                pt.pop(2 * j + 1)
                mk = mt[gi][:, jj, :]
                ps2f = ps2[:].rearrange("l i e -> l (i e)")
                nc.scalar.activation(out=mk[:, 0:384], in_=ps2f[:, 0:384],
                                     func=Copy, bias=0.0, scale=1.0)
                nc.vector.tensor_copy(out=mk[:, 384:768], in_=ps2f[:, 384:768])
                if jj == g - 1:
                    nc.sync.dma_start(out=msgs[:, g0:g0 + g, :],
                                      in_=mt[gi][:])

            for v in range(B2 + 1):
                if v < B2:
                    emit_front(v)
                if v >= 1:
                    emit_back(v - 1)
    nc.finalize()
    return nc


def _get_program(B):
    if B not in _prog_cache:
        _prog_cache[B] = _build_program(B)
    return _prog_cache[B]


def _prepare(x, nodes, fact, params, bias_p, ho_params, ho_bias):
    """Host-side: sort by id, build per-block packed arrays."""
    N, L = nodes.shape
    E = fact.shape[0]
    R = params.shape[2]
    NP = params.shape[0]           # 169
    MA = int(round(NP ** 0.5))     # 13

    ids = (x[fact[:, 0], 1] * MA + x[fact[:, 0], 2]).astype(np.int64)   # [E]
    perm = np.argsort(ids, kind="stable")
    ids_s = ids[perm]
    fact_s = fact[perm].astype(np.int64)                                 # [E,3]

    counts = np.bincount(ids_s, minlength=NP)                            # [NP]
    # one 256-block per id on device; overflow edges (count > 256, ~1.5%)
    # are computed host-side so the device program stays uniform
    dev_counts = np.minimum(counts, _BLK)
    NB = NP                                                              # 169
    B = (NB + _NCORES - 1) // _NCORES
    if B % 2:
        B += 1
    NB8 = B * _NCORES
    blk_ids = np.concatenate([np.arange(NP), np.zeros(NB8 - NB, np.int64)])

    # slot -> sorted-edge-position map (-1 = padding)
    off = np.concatenate([[0], np.cumsum(counts)])
    jloc = np.tile(np.arange(_BLK), NB)
    t_of = np.repeat(np.arange(NP), _BLK)
    src = np.where(jloc < dev_counts[t_of], off[t_of] + jloc, -1)
    src = np.concatenate([src, np.full((NB8 - NB) * _BLK, -1, np.int64)])
    valid = src >= 0

    # overflow edge positions (in sorted order)
    ov_mask = np.zeros(E, bool)
    for tid in np.nonzero(counts > _BLK)[0]:
        ov_mask[off[tid] + _BLK:off[tid + 1]] = True
    ov_pos = np.nonzero(ov_mask)[0]

    # gather features per slot
    nf = nodes[fact_s]                                                   # [E,3,L]
    featp = np.zeros((NB8 * _BLK, 3, L), np.float32)
    featp[valid] = nf[src[valid]]

    # pack pk [8][128, B/2, 896]: even block rows 0:64, odd rows 64:128
    fw = np.zeros((NB8, 64, 896), np.float32)
    fw[:, :, 0:768] = (
        featp.reshape(NB8, _BLK, 3, L).transpose(0, 3, 2, 1)
        .reshape(NB8, 64, 768)
    )
    fw[:, :, 768:896] = params[blk_ids].astype(np.float32)               # W
    fw = _bf16(fw)
    B2 = B // 2
    pk = (fw.reshape(_NCORES, B2, 2, 64, 896).transpose(0, 2, 3, 1, 4)
          .reshape(_NCORES, 128, B2, 896))

    bia = bias_p[blk_ids, 0].astype(np.float32)                          # [NB8,R]
    bia = bia.reshape(_NCORES, B, R).transpose(0, 2, 1)                  # [8,R,B]

    hot = (
        ho_params[:, blk_ids].astype(np.float32).transpose(1, 2, 0, 3)
        .reshape(NB8, R, 3 * L)
    )
    hot = _bf16(hot).reshape(_NCORES, B, R, 192).transpose(0, 2, 1, 3)

    # host path for overflow edges
    msg_ov = None
    if ov_pos.size:
        f_ov = fact_s[ov_pos]                                            # [V,3]
        id_ov = ids_s[ov_pos]                                            # [V]
        W_ov = params[id_ov].astype(np.float32)                          # [V,L,R]
        b_ov = bias_p[id_ov, 0].astype(np.float32)                       # [V,R]
        rn = nodes[f_ov].astype(np.float32)                              # [V,3,L]
        tv = np.maximum(np.einsum('vil,vlr->vir', rn, W_ov) + b_ov[:, None], 0)
        msg_ov = np.empty((ov_pos.size, 3, L), np.float32)
        W2_ov = ho_params[:, id_ov].astype(np.float32)                   # [3,V,R,L]
        for i in range(3):
            j, k2 = [(1, 2), (0, 2), (0, 1)][i]
            pv = tv[:, j] * tv[:, k2]                                    # [V,R]
            msg_ov[:, i] = np.einsum('vr,vrl->vl', pv, W2_ov[i])

    return dict(pk=np.ascontiguousarray(pk),
                bia=np.ascontiguousarray(bia),
                hot=np.ascontiguousarray(hot),
                B=B, NB8=NB8, src=src, valid=valid, fact_s=fact_s,
                ids_s=ids_s, N=N, E=E, L=L, ov_pos=ov_pos, msg_ov=msg_ov)


def _postprocess(msgs_all, prep, ho_bias):
    """Decode per-slot messages, add host-side b2, segment-sum into node_msg."""
    NB8, N, E, L = prep["NB8"], prep["N"], prep["E"], prep["L"]
    src, valid, fact_s, ids_s = prep["src"], prep["valid"], prep["fact_s"], prep["ids_s"]
    B = prep["B"]
    B2 = B // 2
    # msgs_all [8][128, B2, 768] -> [NB8, 64, 768]: row l, col = i*256 + e
    m = (msgs_all.astype(np.float32)
         .reshape(_NCORES, 2, 64, B2, 768).transpose(0, 3, 1, 2, 4)
         .reshape(NB8, 64, 768))
    slots = (
        m.reshape(NB8, 64, 3, _BLK).transpose(0, 3, 2, 1)
        .reshape(NB8 * _BLK, 3, 64)
    )
    msg_e = np.empty((E, 3, L), np.float32)
    msg_e[src[valid]] = slots[valid]
    if prep["msg_ov"] is not None:
        msg_e[prep["ov_pos"]] = prep["msg_ov"]

    # fold in the second bias (linear in the segment-sum)
    msg_e += ho_bias[:, ids_s, 0].astype(np.float32).transpose(1, 0, 2)  # [E,3,L]

    idx_all = fact_s.T.reshape(-1)                                       # [3E]
    val_all = msg_e.transpose(1, 0, 2).reshape(-1, L)                    # [3E,L]
    order = np.argsort(idx_all, kind="stable")
    idx_sorted = idx_all[order]
    val_sorted = val_all[order]
    uniq, starts = np.unique(idx_sorted, return_index=True)
    sums = np.add.reduceat(val_sorted, starts, axis=0)
    out = np.zeros((N, L), np.float32)
    out[uniq] = sums
    return out


def _run_device(prep, trace=False, trace_kwargs=None):
    from concourse.bass_utils import run_bass_kernel_spmd

    B = prep["B"]
    nc = _get_program(B)
    in_maps = []
    for c in range(_NCORES):
        in_maps.append({
            "pk": prep["pk"][c],
            "bia": prep["bia"][c],
            "hot": prep["hot"][c],
        })
    kwargs = {}
    if trace:
        kwargs["trace"] = True
        if trace_kwargs:
            kwargs.update(trace_kwargs)
    res = run_bass_kernel_spmd(nc, in_maps, list(range(_NCORES)), **kwargs)
    msgs_all = np.stack([np.asarray(res.results[c]["msgs"]).astype(np.float32)
                         for c in range(_NCORES)], axis=0)
    return msgs_all, res


def kernel(x, nodes, fact, fact_dim, params, bias_p, ho_params, ho_bias,
           _trace=False, _trace_kwargs=None):
    x = np.asarray(x)
    nodes = np.asarray(nodes, dtype=np.float32)
    fact = np.asarray(fact)
    params = np.asarray(params)
    bias_p = np.asarray(bias_p)
    ho_params = np.asarray(ho_params)
    ho_bias = np.asarray(ho_bias)

    prep = _prepare(x, nodes, fact, params, bias_p, ho_params, ho_bias)
    msgs_all, res = _run_device(prep, trace=_trace, trace_kwargs=_trace_kwargs)
    out = _postprocess(msgs_all, prep, ho_bias)
    kernel.last_results = res
    return out


# revision 27
# speedup vs baseline: 1.0284x; 1.0192x over previous
"""Trainium2 Bass kernel for nn_FGNet (gnn_message_passing), v6.

bf16 + PE array tiling.  Blocks are processed in PAIRS:
  - mm1 (K=64): row-tiled 64x128 -- block 2j's feats/W live on SBUF
    partitions 0:64 (array tile T0), block 2j+1's on 64:128 (T8); the
    two transforms run CONCURRENTLY in the PE array.
  - mm2 (M=64): col-tiled 128x64 -- block 2j's messages land on PSUM
    partitions 0:64 (T0), block 2j+1's on 64:128 (T1), also concurrent.
    One [128,768] cast then drains BOTH blocks' messages.
Bias is applied by the Act engine (bias operand of the relu activation).
Wave-pipelined emission: PE alternates [mm1-pair of wave v] and
[mm2 x6 of wave v-1] so products always have a full wave to land.

Host side: id sort, gather, packing; overflow edges (>256 per id) and
the final segment-sum are computed on host.
"""

import numpy as np

_BLK = 256          # edge slots per block
_NCORES = 8
_GP = 2             # block-pairs per DMA group (4 blocks)

_prog_cache = {}


def _bf16(x):
    import jax.numpy as jnp
    return np.asarray(jnp.asarray(x, dtype=jnp.bfloat16))


def _build_program(B):
    """Device program: B blocks per core, processed as B/2 pairs."""
    import concourse.mybir as mybir
    import concourse.tile as tile
    from concourse import bacc

    F32 = mybir.dt.float32
    BF16 = mybir.dt.bfloat16
    Relu = mybir.ActivationFunctionType.Relu
    Copy = mybir.ActivationFunctionType.Copy

    assert B % 2 == 0
    B2 = B // 2

    nc = bacc.Bacc()
    # pk rows 0:64 = even block, rows 64:128 = odd block of the pair
    # cols 0:768 feats (col = i*256+e), 768:896 = W
    pk = nc.declare_dram_parameter("pk", [128, B2, 896], BF16, isOutput=False)
    bia = nc.declare_dram_parameter("bia", [128, B], F32, isOutput=False)
    hot = nc.declare_dram_parameter("hot", [128, B, 192], BF16, isOutput=False)
    msgs = nc.declare_dram_parameter("msgs", [128, B2, 768], BF16, isOutput=True)

    groups = [(0, 1)]          # small first group -> fast pipeline start
    g0 = 1
    while g0 < B2:
        g = min(_GP, B2 - g0)
        groups.append((g0, g))
        g0 += g
    grp_of = {}
    for gi, (g0, g) in enumerate(groups):
        for j in range(g0, g0 + g):
            grp_of[j] = gi

    with tile.TileContext(nc) as tc:
        with (
            tc.tile_pool(name="const", bufs=1) as const,
            tc.tile_pool(name="work", bufs=3) as work,
            tc.tile_pool(name="blk", bufs=4) as blk,
            tc.tile_pool(name="ps1p", bufs=1, space="PSUM") as ps1p,
            tc.tile_pool(name="ps2p", bufs=2, space="PSUM") as ps2p,
        ):
            ftt, htt, mt = {}, {}, {}
            pt = {}

            def load_group(gi):
                # ft on the Sync DGE ring, ht on GpSimd's -- dispatches and
                # transfers run in parallel instead of one serial queue
                g0, g = groups[gi]
                ft = work.tile([128, g, 896], BF16, name="ft", tag="ft")
                nc.sync.dma_start(out=ft[:], in_=pk[:, g0:g0 + g, :])
                ht = work.tile([128, 2 * g, 192], BF16, name="ht", tag="ht")
                nc.gpsimd.dma_start(out=ht[:], in_=hot[:, 2 * g0:2 * (g0 + g), :])
                mt[gi] = work.tile([128, g, 768], BF16, name="m", tag="m")
                ftt[gi], htt[gi] = ft, ht

            load_group(0)
            bt = const.tile([128, B], F32, name="bt")
            nc.scalar.dma_start(out=bt[:], in_=bia[:])

            def emit_front(j):
                """Row-tiled mm1 pair + relus + products for pair j."""
                gi = grp_of[j]
                g0, _ = groups[gi]
                if j == g0 and gi + 1 < len(groups):
                    load_group(gi + 1)
                ft = ftt[gi]
                jj = j - g0
                ps1a = ps1p.tile([128, 2, 512], F32, name="ps1a", tag="ps1a")
                ps1b = ps1p.tile([128, 2, 512], F32, name="ps1b", tag="ps1b")
                # interleave T0/T8 so the two array tiles run concurrently
                nc.tensor.matmul(out=ps1a[:, 0, 0:384],
                                 lhsT=ft[0:64, jj, 768:896],
                                 rhs=ft[0:64, jj, 0:384],
                                 start=True, stop=True)
                nc.tensor.matmul(out=ps1b[:, 0, 0:384],
                                 lhsT=ft[64:128, jj, 768:896],
                                 rhs=ft[64:128, jj, 0:384],
                                 start=True, stop=True)
                nc.tensor.matmul(out=ps1a[:, 1, 0:384],
                                 lhsT=ft[0:64, jj, 768:896],
                                 rhs=ft[0:64, jj, 384:768],
                                 start=True, stop=True)
                nc.tensor.matmul(out=ps1b[:, 1, 0:384],
                                 lhsT=ft[64:128, jj, 768:896],
                                 rhs=ft[64:128, jj, 384:768],
                                 start=True, stop=True)
                tp = blk.tile([128, 2, 768], BF16, name="tp", tag="tp")
                for h, ps1 in ((0, ps1a), (1, ps1b)):
                    k = 2 * j + h
                    nc.scalar.activation(
                        out=tp[:, h, :].rearrange("r (s c) -> r s c",
                                                  s=2, c=384),
                        in_=ps1[:, :, 0:384],
                        func=Relu, bias=bt[:, k:k + 1], scale=1.0)
                # pair-merged products: one instruction covers both blocks
                p2p = blk.tile([128, 2, 256], BF16, name="p2p", tag="p2p")
                nc.vector.tensor_mul(out=p2p[:], in0=tp[:, :, 0:256],
                                     in1=tp[:, :, 256:512])
                p1p = blk.tile([128, 2, 256], BF16, name="p1p", tag="p1p")
                nc.vector.tensor_mul(out=p1p[:], in0=tp[:, :, 0:256],
                                     in1=tp[:, :, 512:768])
                p0p = blk.tile([128, 2, 256], BF16, name="p0p", tag="p0p")
                nc.gpsimd.tensor_mul(out=p0p[:], in0=tp[:, :, 256:512],
                                     in1=tp[:, :, 512:768])
                pt[j] = (p0p, p1p, p2p)

            def emit_back(j):
                """Col-tiled mm2 x6 + cast + out-DMA for pair j."""
                gi = grp_of[j]
                g0, g = groups[gi]
                jj = j - g0
                ht = htt[gi]
                ps2 = ps2p.tile([128, 3, 256], F32, name="ps2", tag="ps2")
                pp = pt.pop(j)
                for i in (2, 1, 0):
                    for h in (0, 1):
                        nc.tensor.matmul(
                            out=ps2[64 * h:64 * (h + 1), i, :],
                            lhsT=ht[:, 2 * jj + h, 64 * i:64 * (i + 1)],
                            rhs=pp[i][:, h, :],
                            start=True, stop=True)
                mk = mt[gi][:, jj, :]
                ps2f = ps2[:].rearrange("l i e -> l (i e)")
                nc.scalar.activation(out=mk[:, 0:384], in_=ps2f[:, 0:384],
                                     func=Copy, bias=0.0, scale=1.0)
                nc.vector.tensor_copy(out=mk[:, 384:768], in_=ps2f[:, 384:768])
                if jj == g - 1:
                    nc.sync.dma_start(out=msgs[:, g0:g0 + g, :],
                                      in_=mt[gi][:])

            for v in range(B2 + 1):
                if v < B2:
                    emit_front(v)
                if v >= 1:
                    emit_back(v - 1)
    nc.finalize()
    return nc


def _get_program(B):
    if B not in _prog_cache:
        _prog_cache[B] = _build_program(B)
    return _prog_cache[B]


def _prepare(x, nodes, fact, params, bias_p, ho_params, ho_bias):
    """Host-side: sort by id, build per-block packed arrays."""
    N, L = nodes.shape
    E = fact.shape[0]
    R = params.shape[2]
    NP = params.shape[0]           # 169
    MA = int(round(NP ** 0.5))     # 13

    ids = (x[fact[:, 0], 1] * MA + x[fact[:, 0], 2]).astype(np.int64)   # [E]
    perm = np.argsort(ids, kind="stable")
    ids_s = ids[perm]
    fact_s = fact[perm].astype(np.int64)                                 # [E,3]

    counts = np.bincount(ids_s, minlength=NP)                            # [NP]
    # one 256-block per id on device; overflow edges (count > 256, ~1.5%)
    # are computed host-side so the device program stays uniform
    dev_counts = np.minimum(counts, _BLK)
    NB = NP                                                              # 169
    B = (NB + _NCORES - 1) // _NCORES
    if B % 2:
        B += 1
    NB8 = B * _NCORES
    blk_ids = np.concatenate([np.arange(NP), np.zeros(NB8 - NB, np.int64)])

    # slot -> sorted-edge-position map (-1 = padding)
    off = np.concatenate([[0], np.cumsum(counts)])
    jloc = np.tile(np.arange(_BLK), NB)
    t_of = np.repeat(np.arange(NP), _BLK)
    src = np.where(jloc < dev_counts[t_of], off[t_of] + jloc, -1)
    src = np.concatenate([src, np.full((NB8 - NB) * _BLK, -1, np.int64)])
    valid = src >= 0

    # overflow edge positions (in sorted order)
    ov_mask = np.zeros(E, bool)
    for tid in np.nonzero(counts > _BLK)[0]:
        ov_mask[off[tid] + _BLK:off[tid + 1]] = True
    ov_pos = np.nonzero(ov_mask)[0]

    # gather features per slot
    nf = nodes[fact_s]                                                   # [E,3,L]
    featp = np.zeros((NB8 * _BLK, 3, L), np.float32)
    featp[valid] = nf[src[valid]]

    # pack pk [8][128, B/2, 896]: even block rows 0:64, odd rows 64:128
    fw = np.zeros((NB8, 64, 896), np.float32)
    fw[:, :, 0:768] = (
        featp.reshape(NB8, _BLK, 3, L).transpose(0, 3, 2, 1)
        .reshape(NB8, 64, 768)
    )
    fw[:, :, 768:896] = params[blk_ids].astype(np.float32)               # W
    fw = _bf16(fw)
    B2 = B // 2
    pk = (fw.reshape(_NCORES, B2, 2, 64, 896).transpose(0, 2, 3, 1, 4)
          .reshape(_NCORES, 128, B2, 896))

    bia = bias_p[blk_ids, 0].astype(np.float32)                          # [NB8,R]
    bia = bia.reshape(_NCORES, B, R).transpose(0, 2, 1)                  # [8,R,B]

    hot = (
        ho_params[:, blk_ids].astype(np.float32).transpose(1, 2, 0, 3)
        .reshape(NB8, R, 3 * L)
    )
    hot = _bf16(hot).reshape(_NCORES, B, R, 192).transpose(0, 2, 1, 3)

    # host path for overflow edges
    msg_ov = None
    if ov_pos.size:
        f_ov = fact_s[ov_pos]                                            # [V,3]
        id_ov = ids_s[ov_pos]                                            # [V]
        W_ov = params[id_ov].astype(np.float32)                          # [V,L,R]
        b_ov = bias_p[id_ov, 0].astype(np.float32)                       # [V,R]
        rn = nodes[f_ov].astype(np.float32)                              # [V,3,L]
        tv = np.maximum(np.einsum('vil,vlr->vir', rn, W_ov) + b_ov[:, None], 0)
        msg_ov = np.empty((ov_pos.size, 3, L), np.float32)
        W2_ov = ho_params[:, id_ov].astype(np.float32)                   # [3,V,R,L]
        for i in range(3):
            j, k2 = [(1, 2), (0, 2), (0, 1)][i]
            pv = tv[:, j] * tv[:, k2]                                    # [V,R]
            msg_ov[:, i] = np.einsum('vr,vrl->vl', pv, W2_ov[i])

    return dict(pk=np.ascontiguousarray(pk),
                bia=np.ascontiguousarray(bia),
                hot=np.ascontiguousarray(hot),
                B=B, NB8=NB8, src=src, valid=valid, fact_s=fact_s,
                ids_s=ids_s, N=N, E=E, L=L, ov_pos=ov_pos, msg_ov=msg_ov)


def _postprocess(msgs_all, prep, ho_bias):
    """Decode per-slot messages, add host-side b2, segment-sum into node_msg."""
    NB8, N, E, L = prep["NB8"], prep["N"], prep["E"], prep["L"]
    src, valid, fact_s, ids_s = prep["src"], prep["valid"], prep["fact_s"], prep["ids_s"]
    B = prep["B"]
    B2 = B // 2
    # msgs_all [8][128, B2, 768] -> [NB8, 64, 768]: row l, col = i*256 + e
    m = (msgs_all.astype(np.float32)
         .reshape(_NCORES, 2, 64, B2, 768).transpose(0, 3, 1, 2, 4)
         .reshape(NB8, 64, 768))
    slots = (
        m.reshape(NB8, 64, 3, _BLK).transpose(0, 3, 2, 1)
        .reshape(NB8 * _BLK, 3, 64)
    )
    msg_e = np.empty((E, 3, L), np.float32)
    msg_e[src[valid]] = slots[valid]
    if prep["msg_ov"] is not None:
        msg_e[prep["ov_pos"]] = prep["msg_ov"]

    # fold in the second bias (linear in the segment-sum)
    msg_e += ho_bias[:, ids_s, 0].astype(np.float32).transpose(1, 0, 2)  # [E,3,L]

    idx_all = fact_s.T.reshape(-1)                                       # [3E]
    val_all = msg_e.transpose(1, 0, 2).reshape(-1, L)                    # [3E,L]
    order = np.argsort(idx_all, kind="stable")
    idx_sorted = idx_all[order]
    val_sorted = val_all[order]
    uniq, starts = np.unique(idx_sorted, return_index=True)
    sums = np.add.reduceat(val_sorted, starts, axis=0)
    out = np.zeros((N, L), np.float32)
    out[uniq] = sums
    return out


def _run_device(prep, trace=False, trace_kwargs=None):
    from concourse.bass_utils import run_bass_kernel_spmd

    B = prep["B"]
    nc = _get_program(B)
    in_maps = []
    for c in range(_NCORES):
        in_maps.append({
            "pk": prep["pk"][c],
            "bia": prep["bia"][c],
            "hot": prep["hot"][c],
        })
    kwargs = {}
    if trace:
        kwargs["trace"] = True
        if trace_kwargs:
            kwargs.update(trace_kwargs)
    res = run_bass_kernel_spmd(nc, in_maps, list(range(_NCORES)), **kwargs)
    msgs_all = np.stack([np.asarray(res.results[c]["msgs"]).astype(np.float32)
                         for c in range(_NCORES)], axis=0)
    return msgs_all, res


def kernel(x, nodes, fact, fact_dim, params, bias_p, ho_params, ho_bias,
           _trace=False, _trace_kwargs=None):
    x = np.asarray(x)
    nodes = np.asarray(nodes, dtype=np.float32)
    fact = np.asarray(fact)
    params = np.asarray(params)
    bias_p = np.asarray(bias_p)
    ho_params = np.asarray(ho_params)
    ho_bias = np.asarray(ho_bias)

    prep = _prepare(x, nodes, fact, params, bias_p, ho_params, ho_bias)
    msgs_all, res = _run_device(prep, trace=_trace, trace_kwargs=_trace_kwargs)
    out = _postprocess(msgs_all, prep, ho_bias)
    kernel.last_results = res
    return out
